# revision 2
# baseline (speedup 1.0000x reference)
"""MLA Bass kernel for Trainium2, 8 NeuronCores (304109ns, rel err 4.6e-3).

Problem: B=4, S=2048, D=1024, H=16, d_h=64, d_hr=32, d_lat=512, causal,
softmax(scale 1/sqrt(96)), clamp inactive.

Sharding: 8 cores = 4 batches x 2 head-groups of 8 heads.

vs the previous 341902ns version:
  - Phase A (latent down-projections) in bf16 (x, W_DQ/W_DKV/W_KR bf16,
    fp32 PSUM accumulate; measured end-to-end rel err ~4e-3, budget 2e-2).
    Halves the x DMA and the phase-A SBUF footprint.
  - Phase A emits ALL c_Q chunks first, so the whole q-side projection
    (wuqt pairs + rope) starts at ~28us instead of ~74us; k_R, c_KV,
    k-side projections and v follow. xt tiles stay live for the second
    (c_KV) pass (bufs=4).
  - qT tiles get ring addresses disjoint from the phase-A transients, so
    their junk-block zeroing runs at t=0 on gpsimd; kT/v reuse the freed
    phase-A zone (their first writes naturally postdate its release).
  - PV swapped: stationary = p block [128k, 128q], moving = v [128k, 65]
    (64 dims + ones column -> denominator). attn lands [q, d] in PSUM.
  - qb-major PV into single-bank [128, 4*65] quartets; evac+reciprocal
    (DVE) -> per-q-partition scalar multiply (gpsimd) -> PE transpose vs
    identity -> attn^T rebuilt in SBUF. No DRAM roundtrip.
  - Attention runs as a flat two-deep (strip, head) pipeline over four
    512-col q strips; out-projection slabs are woven in as soon as each
    strip's attn^T columns complete. exp (ACT) is the attention pacer and
    ACT carries nothing else during attention.
"""

import math

import ml_dtypes
import numpy as np

B, S, D = 4, 2048, 1024
H, DH, DHR, DLAT = 16, 64, 32, 512
GH = 8  # heads per core group
NCORES = 8
INV_SQRT_DQK = 1.0 / math.sqrt(96.0)

_CACHE = {}


def _rope_tables():
    inv_freq = 10000.0 ** (-np.arange(0, DHR, 2, dtype=np.float64) / DHR)
    ang = np.arange(S, dtype=np.float64)[None, :] * inv_freq[:, None]  # (16, S)
    cos = np.cos(ang).astype(np.float32)
    sin = np.sin(ang).astype(np.float32)
    cosf = np.tile(np.concatenate([cos, cos], axis=0), (4, 1))  # (128, S)
    sinf = np.tile(np.concatenate([-sin, sin], axis=0), (4, 1))  # (128, S)
    return cosf, sinf


def _build(variant="full"):
    import concourse.tile as tile
    from concourse import bacc, mybir

    f32 = mybir.dt.float32
    bf16 = mybir.dt.bfloat16
    Exp = mybir.ActivationFunctionType.Exp

    nc = bacc.Bacc("TRN2", target_bir_lowering=False, debug=False,
                   num_devices=NCORES)

    xT_d = nc.dram_tensor("xT", (D, S), bf16, kind="ExternalInput").ap()
    wdqt_d = nc.dram_tensor("wdqt", (D, DLAT), bf16, kind="ExternalInput").ap()
    wdkvt_d = nc.dram_tensor("wdkvt", (D, DLAT), bf16,
                             kind="ExternalInput").ap()
    wkrt_d = nc.dram_tensor("wkrt", (D, DHR), bf16, kind="ExternalInput").ap()
    wuqt_d = nc.dram_tensor("wuqt", (DLAT, 512), bf16, kind="ExternalInput").ap()
    wqra_d = nc.dram_tensor("wqra", (DLAT, 128), bf16, kind="ExternalInput").ap()
    wqrb_d = nc.dram_tensor("wqrb", (DLAT, 128), bf16, kind="ExternalInput").ap()
    wukt_d = nc.dram_tensor("wukt", (DLAT, 512), bf16, kind="ExternalInput").ap()
    wuvt_d = nc.dram_tensor("wuvt", (DLAT, 512), bf16, kind="ExternalInput").ap()
    wot_d = nc.dram_tensor("wot", (512, D), bf16, kind="ExternalInput").ap()
    cosf_d = nc.dram_tensor("cosf", (128, S), bf16, kind="ExternalInput").ap()
    sinf_d = nc.dram_tensor("sinf", (128, S), bf16, kind="ExternalInput").ap()
    tri_d = nc.dram_tensor("tri", (128, 128), bf16, kind="ExternalInput").ap()
    eye_d = nc.dram_tensor("eye", (128, 128), bf16, kind="ExternalInput").ap()
    ot_d = nc.dram_tensor("ot", (D, S), f32, kind="ExternalOutput").ap()

    swap16 = [(i + 16) % 32 for i in range(32)]

    with tile.TileContext(nc, pool_alloc_mode="queue") as tc:
        re = lambda ap: ap.rearrange("(k p) m -> p k m", p=128)

        # -------- global PSUM pools: 3x2 + 2x1 = 8 banks ------------------
        # pv quartets ([128,512] f32) and transpose outs ([64,1024] bf16)
        # share one slot rotation (same tag)
        work_ps = tc.alloc_tile_pool(name="work_ps", bufs=3, space="PSUM")
        pvtr_ps = tc.alloc_tile_pool(name="pvtr_ps", bufs=2, space="PSUM")

        # SBUF pool releases are LIFO per side. Left stack holds the dying
        # pools, allocated in reverse death order; right stack runs to end.
        p0_pool = tc.alloc_tile_pool(name="p0_pool", bufs=10, side="left")
        ckv_pool = tc.alloc_tile_pool(name="ckv_pool", bufs=1, side="left")
        ckv = ckv_pool.tile([128, 4, S], bf16)
        constsB = tc.alloc_tile_pool(name="constsB", bufs=1, side="left")
        wuqt = constsB.tile([128, 4, 512], bf16, name="wuqt_sb")
        wqra = constsB.tile([128, 4, 128], bf16, name="wqra_sb")
        wqrb = constsB.tile([128, 4, 128], bf16, name="wqrb_sb")
        wukt = constsB.tile([128, 4, 512], bf16, name="wukt_sb")
        wuvt = constsB.tile([128, 4, 512], bf16, name="wuvt_sb")
        cq_pool = tc.alloc_tile_pool(name="cq_pool", bufs=1, side="left")
        cq = cq_pool.tile([128, 4, S], bf16)
        krs_pool = tc.alloc_tile_pool(name="krs_pool", bufs=1, side="left")
        krs = krs_pool.tile([128, S], bf16)  # roped k_R^T at parts [0:32)
        trig = tc.alloc_tile_pool(name="trig", bufs=1, side="left")
        cosf = trig.tile([128, S], bf16, name="cosf_sb")
        sinf = trig.tile([128, S], bf16, name="sinf_sb")
        rope_pool = tc.alloc_tile_pool(name="rope_pool", bufs=1, side="left")
        xt_pool = tc.alloc_tile_pool(name="xt_pool", bufs=4, side="left")
        wdkv_pool = tc.alloc_tile_pool(name="wdkv_pool", bufs=1, side="left")
        wdkvt = wdkv_pool.tile([128, 8, DLAT], bf16, name="wdkvt_sb")
        kr_pool = tc.alloc_tile_pool(name="kr_pool", bufs=1, side="left")
        kr_raw = kr_pool.tile([128, S], bf16, name="kr_raw")
        wkr_pool = tc.alloc_tile_pool(name="wkr_pool", bufs=1, side="left")
        wkrt = wkr_pool.tile([128, 8, DHR], bf16, name="wkrt_sb")
        wdq_pool = tc.alloc_tile_pool(name="wdq_pool", bufs=1, side="left")
        wdqt = wdq_pool.tile([128, 8, DLAT], bf16, name="wdqt_sb")

        constsD = tc.alloc_tile_pool(name="constsD", bufs=1, side="right")
        tri = constsD.tile([128, 128], bf16, name="tri_sb")
        eye = constsD.tile([128, 128], bf16, name="eye_sb")
        qstage_pool = tc.alloc_tile_pool(name="qstage", bufs=3, side="right")
        norm_pool = tc.alloc_tile_pool(name="norm_pool", bufs=3, side="right")
        rcp_pool = tc.alloc_tile_pool(name="rcp_pool", bufs=3, side="right")
        attnT_pool = tc.alloc_tile_pool(name="attnT_pool", bufs=1,
                                        side="right")
        attnT = attnT_pool.tile([128, 4, S], bf16, name="attnT")
        # qT tiles next on the ring: their addresses are disjoint from the
        # phase-A transients, so junk zeroing + q-side writes never wait
        qT0_pool = tc.alloc_tile_pool(name="qT0_pool", bufs=1, side="right")
        qT0 = qT0_pool.tile([128, 4, S], bf16, name="qT0")
        qT1_pool = tc.alloc_tile_pool(name="qT1_pool", bufs=1, side="right")
        qT1 = qT1_pool.tile([128, 4, S], bf16, name="qT1")
        qTs = (qT0, qT1)

        # ---------------- DMA queues ---------------------------------------
        # HWDGE (SP): wdqt + x first, then the late-use weights. SWDGE
        # (gpsimd): the small early-use q-side weights + trig tables.
        xre = xT_d.rearrange("(a p) s -> p a s", p=128)
        xts = []
        for sc in range(4):
            xt = xt_pool.tile([128, 8, 512], bf16, tag="xt", name="xt",
                              bufs=4)
            xts.append(xt)
        def ldk_into(t, dram_ap, eng):
            eng.dma_start(t[:], re(dram_ap))
        # first two k-chunks arrive as small fast DMAs so the first matmul
        # starts at ~2.5us; the rest stream as single large transfers
        nc.sync.dma_start(wdqt[:, 0:2, :], re(wdqt_d)[:, 0:2, :])
        nc.sync.dma_start(xts[0][:, 0:2, :], xre[:, 0:2, 0:512])
        nc.sync.dma_start(wdqt[:, 2:8, :], re(wdqt_d)[:, 2:8, :])
        nc.sync.dma_start(xts[0][:, 2:8, :], xre[:, 2:8, 0:512])
        ldk_into(wuqt, wuqt_d, nc.gpsimd)
        ldk_into(wqra, wqra_d, nc.gpsimd)
        nc.gpsimd.dma_start(cosf[:], cosf_d)
        nc.gpsimd.dma_start(sinf[:], sinf_d)
        ldk_into(wqrb, wqrb_d, nc.gpsimd)
        for sc in range(1, 4):
            nc.sync.dma_start(xts[sc][:], xre[:, :, sc * 512:(sc + 1) * 512])
        ldk_into(wkrt, wkrt_d, nc.sync)
        ldk_into(wdkvt, wdkvt_d, nc.sync)
        ldk_into(wukt, wukt_d, nc.sync)
        ldk_into(wuvt, wuvt_d, nc.sync)
        nc.sync.dma_start(tri[:], tri_d)
        nc.sync.dma_start(eye[:], eye_d)
        for half in range(2):  # qT junk blocks: zero early (gpsimd)
            for hw in range(4):
                jb = slice(96, 128) if hw % 2 == 0 else slice(32, 64)
                nc.gpsimd.memset(qTs[half][jb, hw, :], 0.0)

        # ---------------- phase A part 1: c_Q (bf16) ----------------------
        def down_proj(wt, dst_m_range, dst, sc):
            ssl = slice(sc * 512, (sc + 1) * 512)
            for m in dst_m_range:
                ps = work_ps.tile([128, 1024], f32, tag="wps", name="psa")
                for k in range(8):
                    nc.tensor.matmul(ps[:, 0:512],
                                     wt[:, k, m * 128:(m + 1) * 128],
                                     xts[sc][:, k, :],
                                     start=(k == 0), stop=(k == 7))
                nc.scalar.copy(dst[:, m, ssl], ps[:, 0:512])

        for sc in range(4):
            down_proj(wdqt, range(4), cq, sc)
        wdq_pool.release()

        # ---------------- q-side projections (start ~28us) ----------------
        def proj_pair(j, wsrc, lat, dst):
            for n in range(2):  # 1024-wide S chunks
                ps = work_ps.tile([128, 1024], f32, tag="wps", name="psb")
                for k in range(4):
                    for r_ in range(2):
                        nc.tensor.matmul(
                            ps[:, r_ * 512:(r_ + 1) * 512],
                            wsrc[:, k, j * 128:(j + 1) * 128],
                            lat[:, k, n * 1024 + r_ * 512:n * 1024 + (r_ + 1) * 512],
                            start=(k == 0), stop=(k == 3))
                nsl = slice(n * 1024, (n + 1) * 1024)
                nc.scalar.copy(dst[0:64, 2 * (j % 2), nsl], ps[0:64, :])
                nc.vector.tensor_copy(dst[64:128, 2 * (j % 2) + 1, nsl],
                                      ps[64:128, :])

        def rope_q(wq, heads, qTh):
            for n in range(2):
                ps = work_ps.tile([128, 1024], f32, tag="wps", name="psr")
                for k in range(4):
                    for r_ in range(2):
                        nc.tensor.matmul(
                            ps[:, r_ * 512:(r_ + 1) * 512], wq[:, k, :],
                            cq[:, k, n * 1024 + r_ * 512:n * 1024 + (r_ + 1) * 512],
                            start=(k == 0), stop=(k == 3))
                nsl = slice(n * 1024, (n + 1) * 1024)
                # ACT evacuates PSUM to bf16 so the DVE rope chain gets its
                # 2x/4x bf16 modes
                rb = rope_pool.tile([128, 1024], bf16, tag="rb", name="rb",
                                    bufs=2)
                nc.scalar.copy(rb[:], ps[:])
                swp = rope_pool.tile([128, 1024], bf16, tag="swp", name="swp",
                                     bufs=2)
                nc.vector.stream_shuffle(swp[:], rb[:], swap16)
                t1 = rope_pool.tile([128, 1024], bf16, tag="t1", name="t1")
                nc.vector.tensor_mul(t1[:], rb[:], cosf[:, nsl])
                t2 = rope_pool.tile([128, 1024], bf16, tag="t2", name="t2")
                nc.vector.tensor_mul(t2[:], swp[:], sinf[:, nsl])
                ro = rope_pool.tile([128, 1024], bf16, tag="ro", name="ro")
                nc.vector.tensor_add(ro[:], t1[:], t2[:])
                nc.vector.tensor_copy(qTh[0:32, heads[0], nsl], ro[0:32, :])
                nc.sync.dma_start(qTh[0:32, heads[1], nsl], ro[32:64, :])
                nc.vector.tensor_copy(qTh[64:96, heads[2], nsl], ro[64:96, :])
                nc.sync.dma_start(qTh[64:96, heads[3], nsl], ro[96:128, :])

        for half in range(2):
            for jw in range(2):
                proj_pair(2 * half + jw, wuqt, cq, qTs[half])
            rope_q(wqra if half == 0 else wqrb, (1, 3, 0, 2), qTs[half])

        # ---------------- phase A part 2: k_R, c_KV -----------------------
        for sc in range(4):
            ssl = slice(sc * 512, (sc + 1) * 512)
            ps = work_ps.tile([128, 1024], f32, tag="wps", name="psa")
            for k in range(8):
                nc.tensor.matmul(ps[0:DHR, 0:512], wkrt[:, k, :],
                                 xts[sc][:, k, :],
                                 start=(k == 0), stop=(k == 7))
            nc.scalar.copy(kr_raw[0:DHR, ssl], ps[0:DHR, 0:512])
        wkr_pool.release()
        for n in range(2):  # k_R rope (bf16)
            nsl = slice(n * 1024, (n + 1) * 1024)
            kswp = rope_pool.tile([128, 1024], bf16, tag="swp", name="kswp",
                                  bufs=2)
            nc.vector.stream_shuffle(kswp[0:DHR, :], kr_raw[0:DHR, nsl], swap16)
            kt1 = rope_pool.tile([128, 1024], bf16, tag="t1", name="kt1")
            nc.vector.tensor_mul(kt1[0:DHR, :], kr_raw[0:DHR, nsl],
                                 cosf[0:DHR, nsl])
            kt2 = rope_pool.tile([128, 1024], bf16, tag="t2", name="kt2")
            nc.vector.tensor_mul(kt2[0:DHR, :], kswp[0:DHR, :], sinf[0:DHR, nsl])
            nc.vector.tensor_add(krs[0:DHR, nsl], kt1[0:DHR, :], kt2[0:DHR, :])
        kr_pool.release()
        for sc in range(4):
            down_proj(wdkvt, range(4), ckv, sc)
        wdkv_pool.release()
        xt_pool.release()

        # kT/v tiles allocated now: they reuse the freed phase-A zone (their
        # first writes postdate its release anyway)
        kT0_pool = tc.alloc_tile_pool(name="kT0_pool", bufs=1, side="right")
        kT0 = kT0_pool.tile([128, 4, S], bf16, name="kT0")
        kT1_pool = tc.alloc_tile_pool(name="kT1_pool", bufs=1, side="right")
        kT1 = kT1_pool.tile([128, 4, S], bf16, name="kT1")
        kTs = (kT0, kT1)
        v_pool = tc.alloc_tile_pool(name="v_pool", bufs=1, side="right")
        v_sb = v_pool.tile([128, 16, GH * 65], bf16, name="v_sb")
        nc.gpsimd.memset(  # ones columns first: PV(h0) needs them earliest
            v_sb[:].rearrange("p st (h c) -> p st h c", c=65)[:, :, :, 64:65],
            1.0)
        for half in range(2):  # kT junk zeroing, fine-grained, in use order
            for hw in range(4):
                jb = slice(96, 128) if hw % 2 == 0 else slice(32, 64)
                for scn in range(2):
                    nc.gpsimd.memset(
                        kTs[half][jb, hw, scn * 1024:(scn + 1) * 1024], 0.0)

        # ---------------- k-side projections + v --------------------------
        for half in range(2):
            for jw in range(2):
                proj_pair(2 * half + jw, wukt, ckv, kTs[half])
            for hw in (0, 2):  # even local heads: k rope at [64:96) via DMA
                nc.sync.dma_start(kTs[half][64:96, hw, :], krs[0:DHR, :])
            for hw in (1, 3):  # odd: at [0:32) direct
                nc.vector.tensor_copy(kTs[half][0:DHR, hw, :], krs[0:DHR, :])

        def emit_v(st_range):
            for st in st_range:
                ps = work_ps.tile([128, 1024], f32, tag="wps", name="psv")
                for k in range(4):
                    nc.tensor.matmul(ps[:, 0:512],
                                     ckv[:, k, st * 128:(st + 1) * 128],
                                     wuvt[:, k, :], start=(k == 0), stop=(k == 3))
                dst = v_sb[:, st, :].rearrange("p (h c) -> p h c", c=65)[:, :, 0:64]
                src = ps[:, 0:512].rearrange("p (h c) -> p h c", c=64)
                if st % 2 == 0:
                    nc.scalar.copy(dst, src)
                else:
                    nc.vector.tensor_copy(dst, src)

        emit_v(range(16))
        rope_pool.release()
        trig.release()
        krs_pool.release()
        cq_pool.release()
        constsB.release()
        ckv_pool.release()

        # ---------------- attention ---------------------------------------
        def attn_qk(h, base, width, p_pool):
            kTh = kTs[h // 4][:, h % 4, :]
            qTh = qTs[h // 4][:, h % 4, :]
            mem = []
            for ki in range((base + width) // 128):
                qs = max(base, 128 * ki)
                mem.append((ki, qs, base + width - qs))
            bins = []
            for (ki, qs, w) in sorted(mem, key=lambda m: -m[2]):
                for b_ in bins:
                    if b_[0] + w <= 1024:
                        b_[1].append((ki, qs, w, b_[0]))
                        b_[0] += w
                        break
                else:
                    bins.append([w, [(ki, qs, w, 0)]])
            ploc = {}
            for (used, items) in bins:
                sc_ps = work_ps.tile([128, 1024], f32, tag="wps", name="scp")
                for (ki, qs, w, off) in items:
                    cuts = sorted({off, off + w} | ({512} if off < 512 < off + w
                                                    else set()))
                    for (rs, re_) in zip(cuts, cuts[1:]):
                        nc.tensor.matmul(
                            sc_ps[:, rs:re_],
                            kTh[:, 128 * ki:128 * ki + 128],
                            qTh[:, qs + rs - off:qs + re_ - off],
                            start=True, stop=True)
                p_sb = p_pool.tile([128, 1024], bf16, tag="p", name="p_sb")
                nc.scalar.activation(p_sb[:, 0:used], sc_ps[:, 0:used], Exp,
                                     scale=INV_SQRT_DQK)
                for (ki, qs, w, off) in items:
                    if qs == 128 * ki:  # diagonal block: mask upper triangle
                        nc.vector.tensor_mul(p_sb[:, off:off + 128],
                                             p_sb[:, off:off + 128], tri[:])
                    ploc[ki] = (p_sb, qs, off)
            return (h, base, width, ploc)

        def attn_pv(qk):
            h, base, width, ploc = qk
            nqb = width // 128
            norm = norm_pool.tile([128, 8, 64], bf16, tag="nrm", name="nrm")
            for qt in range(nqb // 4):
                pv = pvtr_ps.tile([128, 512], f32, tag="pvtr", name="pvt")
                for j in range(4):
                    qb = base // 128 + 4 * qt + j
                    for ki in range(qb + 1):
                        p_sb, qs, off = ploc[ki]
                        co = off + (128 * qb - qs)
                        nc.tensor.matmul(
                            pv[:, 65 * j:65 * j + 65],
                            p_sb[:, co:co + 128],
                            v_sb[:, ki, 65 * h:65 * h + 65],
                            start=(ki == 0), stop=(ki == qb))
                pvv = pv[:, 0:260].rearrange("p (a c) -> p a c", c=65)
                rcp = rcp_pool.tile([128, 4], f32, tag="rcp", name="rcp")
                nc.vector.reciprocal(rcp[:], pvv[:, :, 64:65])
                for j in range(4):
                    nc.vector.tensor_scalar_mul(
                        norm[:, 4 * qt + j, :], pvv[:, j, 0:64],
                        rcp[:, j:j + 1])
            return (h, base, width, norm)

        def attn_tail(pend):
            if pend is None:
                return
            h, base, width, norm = pend
            nqb = width // 128
            tr = pvtr_ps.tile([64, 1024], bf16, tag="pvtr", name="trt")
            for j in range(nqb):
                nc.tensor.transpose(tr[:, 128 * j:128 * j + 128],
                                    norm[:, j, :], eye[:])
            nc.vector.tensor_copy(
                attnT[64 * (h % 2):64 * (h % 2) + 64, h // 2,
                      base:base + width],
                tr[:, 0:width])

        def out_slab(base, width, wot, otst_pool, dm_range=range(8), final=False):
            for dm in dm_range:
                ps = work_ps.tile([128, 1024], f32, tag="wps", name="otp")
                for k in range(4):
                    for r_ in range(width // 512):
                        nc.tensor.matmul(
                            ps[:, r_ * 512:(r_ + 1) * 512],
                            wot[:, k, dm * 128:(dm + 1) * 128],
                            attnT[:, k, base + r_ * 512:base + (r_ + 1) * 512],
                            start=(k == 0), stop=(k == 3))
                stg = otst_pool.tile([128, 1024], f32, tag="ot", name="ots")
                if final and dm % 2 == 0:  # tail: ACT is idle, share evacs
                    nc.scalar.copy(stg[:, 0:width], ps[:, 0:width])
                else:
                    nc.vector.tensor_copy(stg[:, 0:width], ps[:, 0:width])
                nc.sync.dma_start(ot_d[dm * 128:(dm + 1) * 128,
                                       base:base + width], stg[:, 0:width])

        # Flat pipeline over (strip, head). lag = how many heads of QK+exp
        # run ahead of PV; deeper lag keeps ACT (exp) from waiting behind
        # PV/transpose work in PE's in-order queue.
        state = {"qks": [], "pends": []}

        def retire_one():
            nxt = attn_pv(state["qks"].pop(0))
            state["pends"].append(nxt)
            if len(state["pends"]) > 1:
                attn_tail(state["pends"].pop(0))

        def step(base, h, pool, lag=1):
            state["qks"].append(attn_qk(h, base, 512, pool))
            if len(state["qks"]) > lag:
                retire_one()

        def drain():
            while state["qks"]:
                retire_one()
            while state["pends"]:
                attn_tail(state["pends"].pop(0))

        for h in range(8):
            step(0, h, p0_pool)
        for h in range(2):
            step(512, h, p0_pool)
        # wot load deferred; lands in freed space
        wot_pool = tc.alloc_tile_pool(name="wot_pool", bufs=1, side="right")
        wot = wot_pool.tile([128, 4, D], bf16, name="wot_sb")
        ldk_into(wot, wot_d, nc.gpsimd)
        otst_pool = tc.alloc_tile_pool(name="otst_pool", bufs=3, side="right")
        step(512, 2, p0_pool)
        out_slab(0, 512, wot, otst_pool, range(0, 4))
        step(512, 3, p0_pool)
        out_slab(0, 512, wot, otst_pool, range(4, 8))
        for h in range(4, 8):
            step(512, h, p0_pool)
        # NOTE: out_slab(base) reads attnT[:, :, base:base+512] for ALL heads;
        # in the Tile dataflow model emission order IS program order, so a
        # slab emitted before attn_tail(strip, h7) reads unwritten data.
        # tail(s, h7) is emitted inside the (lag+1)-th step of the next
        # strip -- slabs may only be emitted after that step.
        p1_pool = tc.alloc_tile_pool(name="p1_pool", bufs=24, side="right")
        step(1024, 0, p1_pool, lag=2)
        step(1024, 1, p1_pool, lag=2)
        step(1024, 2, p1_pool, lag=2)   # emits tail(512, h7)
        out_slab(512, 512, wot, otst_pool, range(0, 4))
        step(1024, 3, p1_pool, lag=2)
        out_slab(512, 512, wot, otst_pool, range(4, 8))
        for h in range(4, 8):
            step(1024, h, p1_pool, lag=2)
        step(1536, 0, p1_pool, lag=2)
        step(1536, 1, p1_pool, lag=2)
        step(1536, 2, p1_pool, lag=2)   # emits tail(1024, h7)
        out_slab(1024, 512, wot, otst_pool, range(0, 4))
        step(1536, 3, p1_pool, lag=2)
        out_slab(1024, 512, wot, otst_pool, range(4, 8))
        for h in range(4, 8):
            step(1536, h, p1_pool, lag=2)
        drain()                         # emits tails (1536, h6) and (1536, h7)
        out_slab(1536, 512, wot, otst_pool, final=True)

        # releases: left stack remainder then right stack (reverse alloc)
        p0_pool.release()
        p1_pool.release()
        otst_pool.release()
        wot_pool.release()
        v_pool.release()
        kT1_pool.release()
        kT0_pool.release()
        qT1_pool.release()
        qT0_pool.release()
        attnT_pool.release()
        rcp_pool.release()
        norm_pool.release()
        qstage_pool.release()
        constsD.release()
        pvtr_ps.release()
        work_ps.release()

    nc.compile()
    return nc


def _get_nc(variant="full"):
    if variant not in _CACHE:
        _CACHE[variant] = _build(variant)
    return _CACHE[variant]


def _prep_inputs(inputs):
    bf = ml_dtypes.bfloat16
    x = np.ascontiguousarray(inputs["x"], dtype=np.float32)
    xT = np.ascontiguousarray(x.transpose(0, 2, 1)).astype(bf)  # (B, D, S)

    wdqt = np.ascontiguousarray(inputs["W_DQ"].T).astype(bf)
    wdkvt = np.ascontiguousarray(inputs["W_DKV"].T).astype(bf)
    perm_eo = np.concatenate([np.arange(0, DHR, 2), np.arange(1, DHR, 2)])
    wkrt = np.ascontiguousarray(inputs["W_KR"][perm_eo, :].T).astype(bf)
    wuqT = np.asarray(inputs["W_UQ"], dtype=np.float32).T  # (512, 1024)
    wukT = np.asarray(inputs["W_UK"], dtype=np.float32).T
    wuvT = np.asarray(inputs["W_UV"], dtype=np.float32).T
    wqr = np.asarray(inputs["W_QR"], dtype=np.float32)  # (512, 512)
    wotT = np.ascontiguousarray(inputs["W_O"].T, dtype=np.float32)

    cosf, sinf = _rope_tables()
    tri = np.triu(np.ones((128, 128), np.float32)).astype(bf)
    eye = np.eye(128, dtype=np.float32).astype(bf)

    in_maps = []
    for core in range(NCORES):
        b, g = core // 2, core % 2
        h0 = GH * g

        def rope_cols(local_heads):
            rows = np.concatenate(
                [(h0 + l) * DHR + perm_eo for l in local_heads])
            return np.ascontiguousarray(wqr[rows, :].T.astype(bf))  # (512, 128)

        in_maps.append({
            "xT": xT[b],
            "wdqt": wdqt,
            "wdkvt": wdkvt,
            "wkrt": wkrt,
            "wuqt": np.ascontiguousarray(
                wuqT[:, h0 * DH:(h0 + GH) * DH].astype(bf)),
            "wqra": rope_cols((1, 3, 0, 2)),
            "wqrb": rope_cols((5, 7, 4, 6)),
            "wukt": np.ascontiguousarray(
                wukT[:, h0 * DH:(h0 + GH) * DH].astype(bf)),
            "wuvt": np.ascontiguousarray(
                wuvT[:, h0 * DH:(h0 + GH) * DH].astype(bf)),
            "wot": np.ascontiguousarray(
                wotT[h0 * DH:(h0 + GH) * DH, :].astype(bf)),
            "cosf": cosf.astype(bf),
            "sinf": sinf.astype(bf),
            "tri": tri,
            "eye": eye,
        })
    return in_maps


def kernel(**inputs):
    from concourse.bass_utils import run_bass_kernel_spmd

    nc = _get_nc()
    in_maps = _prep_inputs(inputs)
    res = run_bass_kernel_spmd(nc, in_maps, core_ids=list(range(NCORES)))
    out = np.empty((B, S, D), dtype=np.float32)
    for b in range(B):
        ot = res.results[2 * b]["ot"] + res.results[2 * b + 1]["ot"]  # (D, S)
        out[b] = ot.T
    return out
